# revision 12
# baseline (speedup 1.0000x reference)
"""Distributed Trainium2 kernel for nn_Attention_40475771797639, v3.

Sharding (unchanged from v2): 8 cores = 4 batches x 2 head-groups (8
heads each).  Each core computes q/k/v projections for its heads over
the FULL sequence, its heads' full S x S attention, and a PARTIAL
output projection; the host sums the pair's partials in assemble().

v3 vs v2 (HW 433-454us):
  1. AV (attn @ v_aug) switched to fp8e4 DoubleRow matmuls: exp writes
     attn weights as fp8, v_aug stored fp8, 2 k-tiles (256 keys) per
     MM at 0.5 cyc/row.  AV PE time 109us -> 27us.  Numerics: CPU sim
     shows rel_err 1.47e-2 (budget 2e-2); quantization ONLY here --
     attention averaging shrinks signal as much as noise, so fp8
     anywhere else (projections, scores) blows the budget (all-fp8
     measured 2.24e-2 on CPU).
  2. Restructured for ACT (exp) saturation: ACT busy ~254us is the
     floor; PE busy ~245us now fits under it.  Loop is qc-outer,
     m-inner; kproj(m+1)/qproj(next)/outproj(qc-1) are emitted as
     explicit bf16 filler between score groups (PE executes in issue
     order, so gap-filling must be interleaved at issue time).  The
     previous (m,qc)'s AV runs as two single-head DR blocks hooked
     after score groups 1 and 3 (paced so the psB pool never stalls
     the in-order PE stream; coarse DR blocks per the v2 lesson).
  3. exp instructions cover 3 PSUM banks (1536 el/lane, last group 2)
     to amortize ACT per-instr overhead: 11 instrs/(m,qc) instead of
     16.  PSUM: scores 2x3 banks + 2 rotating proj/pso banks = 8.
  4. qc0 is PE-oversubscribed (kv proj must complete inside it):
     kproj rides the score gaps, vproj + all 8 deferred per-head AVs
     run at qc0 end (~100us wall).  qc1-3 are ACT-paced (~63us each).

Device layouts (host prep identical to v2: bf16 transposed x/W):
    x0t/x1t/x2t [DIM, S]  wqt/wkt/wvt [DIM, DG]  wpt [DG, DIM]
    bqr/bkr [P,4] f32  bvr [1,DG] bf16  bpr [P,8] f32
    y [DIM, S] f32 partial (host adds pair + transposes)
"""

import numpy as np
import ml_dtypes

B, S, DIM = 4, 2048, 1024
H, DH = 16, 64          # total heads
HG = 8                  # heads per core (head-group)
DG = HG * DH            # 512 local d
SCALE = DH ** -0.5
NCORES = 8
P = 128

_CACHE = {}

# AV matmul mode: "dr" = fp8 DoubleRow (2 k-tiles/MM), "fp8" = normal-mode
# fp8 (same quantized data, 1 k-tile/MM, no PE mode switches)
AV_MODE = "fp8"
# timing-probe knob: emit only the first QC_RUN query chunks (QC_RUN < 4
# gives a WRONG output; used to test how HW time scales with work)
QC_RUN = 4


def build_nc(reps: int = 1):
    import concourse.bacc as bacc
    import concourse.tile as tile
    from concourse import mybir

    f32 = mybir.dt.float32
    bf16 = mybir.dt.bfloat16
    f8 = mybir.dt.float8e4
    AF = mybir.ActivationFunctionType
    DR = mybir.MatmulPerfMode.DoubleRow

    nc = bacc.Bacc(None, target_bir_lowering=False)

    x0t = nc.declare_dram_parameter("x0t", [DIM, S], bf16, isOutput=False)
    x1t = nc.declare_dram_parameter("x1t", [DIM, S], bf16, isOutput=False)
    x2t = nc.declare_dram_parameter("x2t", [DIM, S], bf16, isOutput=False)
    wqt = nc.declare_dram_parameter("wqt", [DIM, DG], bf16, isOutput=False)
    wkt = nc.declare_dram_parameter("wkt", [DIM, DG], bf16, isOutput=False)
    wvt = nc.declare_dram_parameter("wvt", [DIM, DG], bf16, isOutput=False)
    wpt = nc.declare_dram_parameter("wpt", [DG, DIM], bf16, isOutput=False)
    bqr = nc.declare_dram_parameter("bqr", [P, 4], f32, isOutput=False)
    bkr = nc.declare_dram_parameter("bkr", [P, 4], f32, isOutput=False)
    bvr = nc.declare_dram_parameter("bvr", [1, DG], bf16, isOutput=False)
    bpr = nc.declare_dram_parameter("bpr", [P, 8], f32, isOutput=False)
    y = nc.declare_dram_parameter("y", [DIM, S], f32, isOutput=True)

    ET = DIM // P        # 8 contraction tiles for q/k/v proj
    DT = DG // P         # 4 local-d tiles (= head pairs)
    JT = DIM // P        # 8 output tiles for out proj
    KT = S // P          # 16 key-token tiles
    KTP = KT // 2        # 8 key-tile pairs (DoubleRow)
    QC = S // 512        # 4 query chunks
    TC = S // 512        # 4 token chunks
    VW = 80              # padded v_aug row (65 used; 16B-aligned pair stride)

    with tile.TileContext(nc) as tc:
        with (
            tc.tile_pool(name="res", bufs=1) as res,
            tc.tile_pool(name="xq", bufs=2) as xq_pool,
            tc.tile_pool(name="qt", bufs=2) as q_pool,
            tc.tile_pool(name="attn", bufs=2) as attn_pool,
            tc.tile_pool(name="onrm", bufs=2) as onorm_pool,
            tc.tile_pool(name="rec", bufs=2) as rec_pool,
            tc.tile_pool(name="recb", bufs=2) as recb_pool,
            tc.tile_pool(name="ysb", bufs=2) as y_pool,
            tc.tile_pool(name="scor", bufs=2, space="PSUM") as scor_pool,
            tc.tile_pool(name="psB", bufs=2, space="PSUM") as psB_pool,
        ):
            # ---- resident tiles ----
            wq_sb = res.tile([P, ET, DG], bf16, tag="wq")
            wk_sb = res.tile([P, ET, DG], bf16, tag="wk")
            wv_sb = res.tile([P, ET, DG], bf16, tag="wv")
            wp_sb = res.tile([P, DT, DIM], bf16, tag="wp")
            k_sb = res.tile([P, DT, S], bf16, tag="kT")
            x1_sb = res.tile([P, ET, S], bf16, tag="x1r")
            x2_sb = res.tile([P, ET, S], bf16, tag="x2r")
            vaug8 = res.tile([P, KTP, 2, HG, VW], f8, tag="vaug")
            bq_sb = res.tile([P, 4], f32, tag="bq")
            bk_sb = res.tile([P, 4], f32, tag="bk")
            bp_sb = res.tile([P, 8], f32, tag="bp")
            bv_sb = res.tile([P, DG], bf16, tag="bv")

            wqt_r = wqt.rearrange("(et p) d -> p et d", p=P)
            wkt_r = wkt.rearrange("(et p) d -> p et d", p=P)
            wvt_r = wvt.rearrange("(et p) d -> p et d", p=P)
            wpt_r = wpt.rearrange("(dt p) j -> p dt j", p=P)
            for et in range(ET):
                nc.sync.dma_start(out=wq_sb[:, et, :], in_=wqt_r[:, et, :])
                nc.sync.dma_start(out=wk_sb[:, et, :], in_=wkt_r[:, et, :])
                nc.sync.dma_start(out=wv_sb[:, et, :], in_=wvt_r[:, et, :])
            for dt in range(DT):
                nc.sync.dma_start(out=wp_sb[:, dt, :], in_=wpt_r[:, dt, :])
            nc.sync.dma_start(out=bq_sb, in_=bqr[:, :])
            nc.sync.dma_start(out=bk_sb, in_=bkr[:, :])
            nc.sync.dma_start(out=bp_sb, in_=bpr[:, :])
            nc.gpsimd.dma_start(out=bv_sb, in_=bvr[:, :].to_broadcast([P, DG]))
            # softmax-denominator ones column per head in v_aug
            nc.vector.memset(vaug8[:, :, :, :, DH], 1.0)

            x0t_r = x0t.rearrange("(et p) t -> p et t", p=P)
            x1t_r = x1t.rearrange("(et p) t -> p et t", p=P)
            x2t_r = x2t.rearrange("(et p) t -> p et t", p=P)
            y_r = y.rearrange("(jt p) t -> p jt t", p=P)

            for rep in range(reps):
                # ---------- per-rep input load (x1/x2 resident) ----------
                for eh in range(4):
                    nc.sync.dma_start(
                        out=x1_sb[:, 2 * eh:2 * eh + 2, :],
                        in_=x1t_r[:, 2 * eh:2 * eh + 2, :])
                    nc.sync.dma_start(
                        out=x2_sb[:, 2 * eh:2 * eh + 2, :],
                        in_=x2t_r[:, 2 * eh:2 * eh + 2, :])

                # ---------- emission helpers ----------

                def load_xq(c):
                    xt = xq_pool.tile([P, ET, 512], bf16, tag="xq", name="xq")
                    for eh in range(4):
                        nc.sync.dma_start(
                            out=xt[:, 2 * eh:2 * eh + 2, :],
                            in_=x0t_r[:, 2 * eh:2 * eh + 2, c * 512:(c + 1) * 512])
                    return xt

                def wproj_unit(w_sb, b_sb, m, rhs_x, c0, out_ap):
                    # k/q projection for head-pair m over token cols
                    # [c0, c0+512): 2 subunits of 4 MMs; bias drain on 2nd.
                    ps = [None]

                    def sub_a():
                        ps[0] = psB_pool.tile([P, 512], f32, tag="psB", name="psu")
                        for et in range(4):
                            nc.tensor.matmul(
                                ps[0],
                                lhsT=w_sb[:, et, m * P:(m + 1) * P],
                                rhs=rhs_x[:, et, c0:c0 + 512],
                                start=(et == 0), stop=False)

                    def sub_b():
                        for et in range(4, ET):
                            nc.tensor.matmul(
                                ps[0],
                                lhsT=w_sb[:, et, m * P:(m + 1) * P],
                                rhs=rhs_x[:, et, c0:c0 + 512],
                                start=False, stop=(et == ET - 1))
                        nc.vector.tensor_scalar_add(out_ap, ps[0], b_sb[:, m:m + 1])

                    return [sub_a, sub_b]

                def vproj_unit(tt):
                    # v[tt keys, all local d] -> fp8 v_aug; resident x2 chunk
                    # as stationary
                    ps = [None]
                    toff = tt * P

                    def sub_a():
                        ps[0] = psB_pool.tile([P, 512], f32, tag="psB", name="psv")
                        for et in range(4):
                            nc.tensor.matmul(
                                ps[0],
                                lhsT=x2_sb[:, et, toff:toff + P],
                                rhs=wv_sb[:, et, :],
                                start=(et == 0), stop=False)

                    def sub_b():
                        for et in range(4, ET):
                            nc.tensor.matmul(
                                ps[0],
                                lhsT=x2_sb[:, et, toff:toff + P],
                                rhs=wv_sb[:, et, :],
                                start=False, stop=(et == ET - 1))
                        nc.vector.tensor_add(
                            out=vaug8[:, tt // 2, tt % 2, :, 0:DH],
                            in0=ps[0].rearrange("p (h d) -> p h d", d=DH),
                            in1=bv_sb.rearrange("p (h d) -> p h d", d=DH))

                    return [sub_a, sub_b]

                def outproj_unit(qc, jt, onorm):
                    def sub():
                        ps = psB_pool.tile([P, 512], f32, tag="psB", name="psj")
                        for dt in range(DT):
                            nc.tensor.matmul(
                                ps,
                                lhsT=wp_sb[:, dt, jt * P:(jt + 1) * P],
                                rhs=onorm[:, dt, :],
                                start=(dt == 0), stop=(dt == DT - 1))
                        ysb = y_pool.tile([P, 512], f32, tag="ysb", name="ysb")
                        nc.vector.tensor_scalar_add(ysb, ps, bp_sb[:, jt:jt + 1])
                        nc.sync.dma_start(
                            out=y_r[:, jt, qc * 512:(qc + 1) * 512], in_=ysb)

                    return sub

                def av_head(m, i, attn, onorm):
                    # one head's AV block (fp8) + normalization
                    def run():
                        h = 2 * m + i
                        pso = psB_pool.tile([P, 512], f32, tag="psB", name="pso")
                        if AV_MODE == "dr":
                            for ktp in range(KTP):
                                nc.tensor.matmul(
                                    pso[0:DH + 1, :],
                                    lhsT=vaug8[:, ktp, :, h, 0:DH + 1],
                                    rhs=attn[:, ktp, :, i, :],
                                    start=(ktp == 0), stop=(ktp == KTP - 1),
                                    perf_mode=DR)
                        else:
                            for kt in range(KT):
                                nc.tensor.matmul(
                                    pso[0:DH + 1, :],
                                    lhsT=vaug8[:, kt // 2, kt % 2, h, 0:DH + 1],
                                    rhs=attn[:, kt // 2, kt % 2, i, :],
                                    start=(kt == 0), stop=(kt == KT - 1))
                        rec = rec_pool.tile([1, 512], f32, tag="rec", name="rec")
                        nc.vector.reciprocal(rec, pso[DH:DH + 1, :])
                        recb = recb_pool.tile([P, 512], f32, tag="recb", name="recb")
                        nc.gpsimd.partition_broadcast(recb, rec)
                        doff = i * DH
                        nc.vector.tensor_mul(
                            out=onorm[doff:doff + DH, m, :],
                            in0=pso[0:DH, :],
                            in1=recb[doff:doff + DH, :])

                    return run

                def scores_exp(m, qt, fillers, hooks):
                    """11 exp groups (3x10 + 2 PSUM banks).  bf16 filler
                    subunits are popped between groups; hooks[g] (single-head
                    AV blocks of the previous (m,qc)) run after group g."""
                    attn = attn_pool.tile([P, KTP, 2, 2, 512], f8, tag="attn",
                                          name="attn")
                    attn_flat = attn.rearrange("p a b c q -> p (a b c q)")
                    tau = 0
                    for g in range(11):
                        sz = 3 if g < 10 else 2
                        st = scor_pool.tile([P, 3, 512], f32, tag="scor",
                                            name="st")
                        for j in range(sz):
                            tj = tau + j
                            ktp, mem, i = tj // 4, (tj // 2) % 2, tj % 2
                            kt = 2 * ktp + mem
                            doff = i * DH
                            nc.tensor.matmul(
                                st[:, j, :],
                                lhsT=k_sb[doff:doff + DH, m, kt * P:(kt + 1) * P],
                                rhs=qt[doff:doff + DH, :],
                                start=True, stop=True)
                        nc.scalar.activation(
                            attn_flat[:, tau * 512:(tau + sz) * 512],
                            st[:, 0:sz, :], AF.Exp, scale=-SCALE)
                        tau += sz
                        if g in hooks:
                            hooks[g]()
                        elif fillers:
                            fillers.pop(0)()
                    for f_ in fillers:   # leftovers run inline after the phase
                        f_()
                    del fillers[:]
                    return attn

                # ---------- per-rep schedule ----------
                pend_avs = []            # single-head AV closures, 1 phase back
                onorm_by_qc = {}
                xq_cur = load_xq(0)
                qt_next = None

                for qc in range(QC_RUN):
                    onorm = onorm_pool.tile([P, DT, 512], bf16, tag="onrm",
                                            name="onorm")
                    onorm_by_qc[qc] = onorm
                    for m in range(DT):
                        if m == 0 and qc == 0:
                            # serial head: kproj(m0), vproj(all), qproj(m0,0)
                            for t in range(TC):
                                for s_ in wproj_unit(
                                        wk_sb, bk_sb, 0, x1_sb, t * 512,
                                        k_sb[:, 0, t * 512:(t + 1) * 512]):
                                    s_()
                            for tt in range(KT):
                                for s_ in vproj_unit(tt):
                                    s_()
                            qt = q_pool.tile([P, 512], bf16, tag="qt",
                                             name="qt0")
                            for s_ in wproj_unit(wq_sb, bq_sb, 0, xq_cur, 0,
                                                 qt):
                                s_()
                        else:
                            qt = qt_next

                        # ---- fillers for this scores phase ----
                        fillers = []
                        # qproj of the NEXT (m, qc) first (its affine must
                        # land before the next phase's first score MM)
                        nm, nqc = (m + 1, qc) if m < DT - 1 else (0, qc + 1)
                        if nqc < QC_RUN:
                            if nm == 0:
                                xq_cur = load_xq(nqc)
                            qt_next = q_pool.tile([P, 512], bf16, tag="qt",
                                                  name="qtn")
                            fillers.extend(
                                wproj_unit(wq_sb, bq_sb, nm, xq_cur, 0,
                                           qt_next))
                        if qc == 0:
                            if m < DT - 1:
                                for t in range(TC):
                                    fillers.extend(wproj_unit(
                                        wk_sb, bk_sb, m + 1, x1_sb, t * 512,
                                        k_sb[:, m + 1, t * 512:(t + 1) * 512]))
                        else:
                            # outproj(qc-1) rides m1/m2 gaps: by then its
                            # onorm is fully written (m0's hooks still write
                            # onorm[., 3, :] of the previous qc)
                            if m == 1:
                                for jt in range(0, 4):
                                    fillers.append(
                                        outproj_unit(qc - 1, jt, onorm_by_qc[qc - 1]))
                            elif m == 2:
                                for jt in range(4, JT):
                                    fillers.append(
                                        outproj_unit(qc - 1, jt, onorm_by_qc[qc - 1]))

                        # ---- AV hooks: previous (m,qc)'s two heads ----
                        hooks = {}
                        if pend_avs:
                            hooks[1] = pend_avs.pop(0)
                            hooks[3] = pend_avs.pop(0)

                        attn = scores_exp(m, qt, fillers, hooks)
                        pend_avs.append(av_head(m, 0, attn, onorm))
                        pend_avs.append(av_head(m, 1, attn, onorm))

                # tail: last AV pair + out projection of qc3
                for run in pend_avs:
                    run()
                pend_avs = []
                for jt in range(JT):
                    outproj_unit(QC_RUN - 1, jt, onorm_by_qc[QC_RUN - 1])()

    nc.compile()
    return nc


def make_in_maps(x0, x1, x2, Wq, bq, Wk, bk, Wv, bv, Wp, bp):
    """Host-side shard prep: per-core transposed bf16 views."""
    bf = ml_dtypes.bfloat16
    zeros_bp = np.zeros_like(bp)
    xts = []
    for b in range(B):
        xts.append(
            (
                np.ascontiguousarray(x0[b].T).astype(bf),
                np.ascontiguousarray(x1[b].T).astype(bf),
                np.ascontiguousarray(x2[b].T).astype(bf),
            )
        )
    gparts = []
    for g in range(2):
        sl = slice(g * DG, (g + 1) * DG)
        bp_g = bp if g == 0 else zeros_bp
        gparts.append(
            {
                "wqt": np.ascontiguousarray(Wq[sl, :].T).astype(bf),
                "wkt": np.ascontiguousarray(Wk[sl, :].T).astype(bf),
                "wvt": np.ascontiguousarray(Wv[sl, :].T).astype(bf),
                "wpt": np.ascontiguousarray(Wp[:, sl].T).astype(bf),
                "bqr": np.ascontiguousarray(bq[sl].reshape(4, P).T).astype(np.float32),
                "bkr": np.ascontiguousarray(bk[sl].reshape(4, P).T).astype(np.float32),
                "bvr": bv[sl].reshape(1, DG).astype(bf),
                "bpr": np.ascontiguousarray(bp_g.reshape(8, P).T).astype(np.float32),
            }
        )
    in_maps = []
    for c in range(NCORES):
        b, g = c // 2, c % 2
        x0t_b, x1t_b, x2t_b = xts[b]
        m = {"x0t": x0t_b, "x1t": x1t_b, "x2t": x2t_b}
        m.update(gparts[g])
        in_maps.append(m)
    return in_maps


def assemble(results):
    out = np.empty((B, S, DIM), np.float32)
    for b in range(B):
        yp = results[2 * b]["y"] + results[2 * b + 1]["y"]
        out[b] = yp.T
    return out


def kernel(**inputs):
    from concourse.bass_utils import run_bass_kernel_spmd

    if "nc" not in _CACHE:
        _CACHE["nc"] = build_nc()
    nc = _CACHE["nc"]
    in_maps = make_in_maps(**inputs)
    res = run_bass_kernel_spmd(nc, in_maps, list(range(NCORES)))
    return assemble([r for r in res.results])


# revision 14
# speedup vs baseline: 1.2813x; 1.2813x over previous
"""Distributed Trainium2 kernel for nn_Attention_40475771797639, v3.

Sharding (unchanged from v2): 8 cores = 4 batches x 2 head-groups (8
heads each).  Each core computes q/k/v projections for its heads over
the FULL sequence, its heads' full S x S attention, and a PARTIAL
output projection; the host sums the pair's partials in assemble().

v3 vs v2 (HW 433-454us):
  1. AV (attn @ v_aug) switched to fp8e4 DoubleRow matmuls: exp writes
     attn weights as fp8, v_aug stored fp8, 2 k-tiles (256 keys) per
     MM at 0.5 cyc/row.  AV PE time 109us -> 27us.  Numerics: CPU sim
     shows rel_err 1.47e-2 (budget 2e-2); quantization ONLY here --
     attention averaging shrinks signal as much as noise, so fp8
     anywhere else (projections, scores) blows the budget (all-fp8
     measured 2.24e-2 on CPU).
  2. Restructured for ACT (exp) saturation: ACT busy ~254us is the
     floor; PE busy ~245us now fits under it.  Loop is qc-outer,
     m-inner; kproj(m+1)/qproj(next)/outproj(qc-1) are emitted as
     explicit bf16 filler between score groups (PE executes in issue
     order, so gap-filling must be interleaved at issue time).  The
     previous (m,qc)'s AV runs as two single-head DR blocks hooked
     after score groups 1 and 3 (paced so the psB pool never stalls
     the in-order PE stream; coarse DR blocks per the v2 lesson).
  3. exp instructions cover 3 PSUM banks (1536 el/lane, last group 2)
     to amortize ACT per-instr overhead: 11 instrs/(m,qc) instead of
     16.  PSUM: scores 2x3 banks + 2 rotating proj/pso banks = 8.
  4. qc0 is PE-oversubscribed (kv proj must complete inside it):
     kproj rides the score gaps, vproj + all 8 deferred per-head AVs
     run at qc0 end (~100us wall).  qc1-3 are ACT-paced (~63us each).

Device layouts (host prep identical to v2: bf16 transposed x/W):
    x0t/x1t/x2t [DIM, S]  wqt/wkt/wvt [DIM, DG]  wpt [DG, DIM]
    bqr/bkr [P,4] f32  bvr [1,DG] bf16  bpr [P,8] f32
    y [DIM, S] f32 partial (host adds pair + transposes)
"""

import numpy as np
import ml_dtypes

B, S, DIM = 4, 2048, 1024
H, DH = 16, 64          # total heads
HG = 8                  # heads per core (head-group)
DG = HG * DH            # 512 local d
SCALE = DH ** -0.5
NCORES = 8
P = 128

_CACHE = {}

# AV matmul mode: "dr" = fp8 DoubleRow (2 k-tiles/MM), "fp8" = normal-mode
# fp8 (same quantized data, 1 k-tile/MM, no PE mode switches)
AV_MODE = "fp8"
# timing-probe knob: emit only the first QC_RUN query chunks (QC_RUN < 4
# gives a WRONG output; used to test how HW time scales with work)
QC_RUN = 4
# elimination probes (WRONG output; timing only): "" | "nonorm" | "smallexp"
PROBE = ""


def build_nc(reps: int = 1):
    import concourse.bacc as bacc
    import concourse.tile as tile
    from concourse import mybir

    f32 = mybir.dt.float32
    bf16 = mybir.dt.bfloat16
    f8 = mybir.dt.float8e4
    AF = mybir.ActivationFunctionType
    DR = mybir.MatmulPerfMode.DoubleRow

    nc = bacc.Bacc(None, target_bir_lowering=False)

    x0t = nc.declare_dram_parameter("x0t", [DIM, S], bf16, isOutput=False)
    x1t = nc.declare_dram_parameter("x1t", [DIM, S], bf16, isOutput=False)
    x2t = nc.declare_dram_parameter("x2t", [DIM, S], bf16, isOutput=False)
    wqt = nc.declare_dram_parameter("wqt", [DIM, DG], bf16, isOutput=False)
    wkt = nc.declare_dram_parameter("wkt", [DIM, DG], bf16, isOutput=False)
    wvt = nc.declare_dram_parameter("wvt", [DIM, DG], bf16, isOutput=False)
    wpt = nc.declare_dram_parameter("wpt", [DG, DIM], bf16, isOutput=False)
    bqr = nc.declare_dram_parameter("bqr", [P, 4], f32, isOutput=False)
    bkr = nc.declare_dram_parameter("bkr", [P, 4], f32, isOutput=False)
    bvr = nc.declare_dram_parameter("bvr", [1, DG], bf16, isOutput=False)
    bpr = nc.declare_dram_parameter("bpr", [P, 8], f32, isOutput=False)
    y = nc.declare_dram_parameter("y", [DIM, S], f32, isOutput=True)

    ET = DIM // P        # 8 contraction tiles for q/k/v proj
    DT = DG // P         # 4 local-d tiles (= head pairs)
    JT = DIM // P        # 8 output tiles for out proj
    KT = S // P          # 16 key-token tiles
    KTP = KT // 2        # 8 key-tile pairs (DoubleRow)
    QC = S // 512        # 4 query chunks
    TC = S // 512        # 4 token chunks
    VW = 80              # padded v_aug row (65 used; 16B-aligned pair stride)

    with tile.TileContext(nc) as tc:
        with (
            tc.tile_pool(name="res", bufs=1) as res,
            tc.tile_pool(name="xq", bufs=2) as xq_pool,
            tc.tile_pool(name="qt", bufs=2) as q_pool,
            tc.tile_pool(name="attn", bufs=2) as attn_pool,
            tc.tile_pool(name="onrm", bufs=2) as onorm_pool,
            tc.tile_pool(name="rec", bufs=2) as rec_pool,
            tc.tile_pool(name="ou", bufs=2) as ou_pool,
            tc.tile_pool(name="recb", bufs=2) as recb_pool,
            tc.tile_pool(name="ysb", bufs=2) as y_pool,
            tc.tile_pool(name="scor", bufs=2, space="PSUM") as scor_pool,
            tc.tile_pool(name="psB", bufs=2, space="PSUM") as psB_pool,
        ):
            # ---- resident tiles ----
            wq_sb = res.tile([P, ET, DG], bf16, tag="wq")
            wk_sb = res.tile([P, ET, DG], bf16, tag="wk")
            wv_sb = res.tile([P, ET, DG], bf16, tag="wv")
            wp_sb = res.tile([P, DT, DIM], bf16, tag="wp")
            k_sb = res.tile([P, DT, S], bf16, tag="kT")
            x1_sb = res.tile([P, ET, S], bf16, tag="x1r")
            x2_sb = res.tile([P, ET, S], bf16, tag="x2r")
            vaug8 = res.tile([P, KTP, 2, HG, VW], f8, tag="vaug")
            bq_sb = res.tile([P, 4], f32, tag="bq")
            bk_sb = res.tile([P, 4], f32, tag="bk")
            bp_sb = res.tile([P, 8], f32, tag="bp")
            bv_sb = res.tile([P, DG], bf16, tag="bv")

            wqt_r = wqt.rearrange("(et p) d -> p et d", p=P)
            wkt_r = wkt.rearrange("(et p) d -> p et d", p=P)
            wvt_r = wvt.rearrange("(et p) d -> p et d", p=P)
            wpt_r = wpt.rearrange("(dt p) j -> p dt j", p=P)
            for et in range(ET):
                nc.sync.dma_start(out=wq_sb[:, et, :], in_=wqt_r[:, et, :])
                nc.sync.dma_start(out=wk_sb[:, et, :], in_=wkt_r[:, et, :])
                nc.sync.dma_start(out=wv_sb[:, et, :], in_=wvt_r[:, et, :])
            for dt in range(DT):
                nc.sync.dma_start(out=wp_sb[:, dt, :], in_=wpt_r[:, dt, :])
            nc.sync.dma_start(out=bq_sb, in_=bqr[:, :])
            nc.sync.dma_start(out=bk_sb, in_=bkr[:, :])
            nc.sync.dma_start(out=bp_sb, in_=bpr[:, :])
            nc.gpsimd.dma_start(out=bv_sb, in_=bvr[:, :].to_broadcast([P, DG]))
            # softmax-denominator ones column per head in v_aug
            nc.vector.memset(vaug8[:, :, :, :, DH], 1.0)

            x0t_r = x0t.rearrange("(et p) t -> p et t", p=P)
            x1t_r = x1t.rearrange("(et p) t -> p et t", p=P)
            x2t_r = x2t.rearrange("(et p) t -> p et t", p=P)
            y_r = y.rearrange("(jt p) t -> p jt t", p=P)

            for rep in range(reps):
                # ---------- per-rep input load (x1/x2 resident) ----------
                for eh in range(4):
                    nc.sync.dma_start(
                        out=x1_sb[:, 2 * eh:2 * eh + 2, :],
                        in_=x1t_r[:, 2 * eh:2 * eh + 2, :])
                    nc.sync.dma_start(
                        out=x2_sb[:, 2 * eh:2 * eh + 2, :],
                        in_=x2t_r[:, 2 * eh:2 * eh + 2, :])

                # ---------- emission helpers ----------

                def load_xq(c):
                    xt = xq_pool.tile([P, ET, 512], bf16, tag="xq", name="xq")
                    for eh in range(4):
                        nc.sync.dma_start(
                            out=xt[:, 2 * eh:2 * eh + 2, :],
                            in_=x0t_r[:, 2 * eh:2 * eh + 2, c * 512:(c + 1) * 512])
                    return xt

                def wproj_unit(w_sb, b_sb, m, rhs_x, c0, out_ap):
                    # k/q projection for head-pair m over token cols
                    # [c0, c0+512): 2 subunits of 4 MMs; bias drain on 2nd.
                    ps = [None]

                    def sub_a():
                        ps[0] = psB_pool.tile([P, 512], f32, tag="psB", name="psu")
                        for et in range(4):
                            nc.tensor.matmul(
                                ps[0],
                                lhsT=w_sb[:, et, m * P:(m + 1) * P],
                                rhs=rhs_x[:, et, c0:c0 + 512],
                                start=(et == 0), stop=False)

                    def sub_b():
                        for et in range(4, ET):
                            nc.tensor.matmul(
                                ps[0],
                                lhsT=w_sb[:, et, m * P:(m + 1) * P],
                                rhs=rhs_x[:, et, c0:c0 + 512],
                                start=False, stop=(et == ET - 1))
                        nc.vector.tensor_scalar_add(out_ap, ps[0], b_sb[:, m:m + 1])

                    return [sub_a, sub_b]

                def vproj_unit(tt):
                    # v[tt keys, all local d] -> fp8 v_aug; resident x2 chunk
                    # as stationary
                    ps = [None]
                    toff = tt * P

                    def sub_a():
                        ps[0] = psB_pool.tile([P, 512], f32, tag="psB", name="psv")
                        for et in range(4):
                            nc.tensor.matmul(
                                ps[0],
                                lhsT=x2_sb[:, et, toff:toff + P],
                                rhs=wv_sb[:, et, :],
                                start=(et == 0), stop=False)

                    def sub_b():
                        for et in range(4, ET):
                            nc.tensor.matmul(
                                ps[0],
                                lhsT=x2_sb[:, et, toff:toff + P],
                                rhs=wv_sb[:, et, :],
                                start=False, stop=(et == ET - 1))
                        nc.vector.tensor_add(
                            out=vaug8[:, tt // 2, tt % 2, :, 0:DH],
                            in0=ps[0].rearrange("p (h d) -> p h d", d=DH),
                            in1=bv_sb.rearrange("p (h d) -> p h d", d=DH))

                    return [sub_a, sub_b]

                def outproj_unit(qc, jt, onorm):
                    def sub():
                        ps = psB_pool.tile([P, 512], f32, tag="psB", name="psj")
                        for dt in range(DT):
                            nc.tensor.matmul(
                                ps,
                                lhsT=wp_sb[:, dt, jt * P:(jt + 1) * P],
                                rhs=onorm[:, dt, :],
                                start=(dt == 0), stop=(dt == DT - 1))
                        ysb = y_pool.tile([P, 512], f32, tag="ysb", name="ysb")
                        nc.vector.tensor_scalar_add(ysb, ps, bp_sb[:, jt:jt + 1])
                        nc.sync.dma_start(
                            out=y_r[:, jt, qc * 512:(qc + 1) * 512], in_=ysb)

                    return sub

                def av_head(m, i, attn, onorm):
                    # one head's AV block (fp8) + normalization
                    def run():
                        h = 2 * m + i
                        pso = psB_pool.tile([P, 512], f32, tag="psB", name="pso")
                        if AV_MODE == "dr":
                            for ktp in range(KTP):
                                nc.tensor.matmul(
                                    pso[0:DH + 1, :],
                                    lhsT=vaug8[:, ktp, :, h, 0:DH + 1],
                                    rhs=attn[:, ktp, :, i, :],
                                    start=(ktp == 0), stop=(ktp == KTP - 1),
                                    perf_mode=DR)
                        else:
                            for kt in range(KT):
                                nc.tensor.matmul(
                                    pso[0:DH + 1, :],
                                    lhsT=vaug8[:, kt // 2, kt % 2, h, 0:DH + 1],
                                    rhs=attn[:, kt // 2, kt % 2, i, :],
                                    start=(kt == 0), stop=(kt == KT - 1))
                        doff = i * DH
                        if PROBE == "nonorm":
                            nc.vector.tensor_copy(
                                out=onorm[doff:doff + DH, m, :],
                                in_=pso[0:DH, :])
                        else:
                            # copy psum -> SBUF first: psB slot frees after
                            # one cheap DVE op instead of after the whole
                            # recip -> gpsimd-broadcast -> mul chain (whose
                            # ~4-5us latency otherwise stalls the 2-slot psB
                            # rotation and with it the in-order PE stream)
                            ou = ou_pool.tile([DH + 1, 512], f32, tag="ou",
                                              name="ou")
                            nc.vector.tensor_copy(out=ou, in_=pso[0:DH + 1, :])
                            rec = rec_pool.tile([1, 512], f32, tag="rec", name="rec")
                            nc.vector.reciprocal(rec, ou[DH:DH + 1, :])
                            recb = recb_pool.tile([DH, 512], f32, tag="recb", name="recb")
                            nc.gpsimd.partition_broadcast(recb, rec)
                            nc.vector.tensor_mul(
                                out=onorm[doff:doff + DH, m, :],
                                in0=ou[0:DH, :],
                                in1=recb[0:DH, :])

                    return run

                def scores_exp(m, qt, fillers, hooks):
                    """11 exp groups (3x10 + 2 PSUM banks).  bf16 filler
                    subunits are popped between groups; hooks[g] (single-head
                    AV blocks of the previous (m,qc)) run after group g."""
                    attn = attn_pool.tile([P, KTP, 2, 2, 512], f8, tag="attn",
                                          name="attn")
                    attn_flat = attn.rearrange("p a b c q -> p (a b c q)")
                    tau = 0
                    for g in range(11):
                        sz = 3 if g < 10 else 2
                        st = scor_pool.tile([P, 3, 512], f32, tag="scor",
                                            name="st")
                        for j in range(sz):
                            tj = tau + j
                            ktp, mem, i = tj // 4, (tj // 2) % 2, tj % 2
                            kt = 2 * ktp + mem
                            doff = i * DH
                            nc.tensor.matmul(
                                st[:, j, :],
                                lhsT=k_sb[doff:doff + DH, m, kt * P:(kt + 1) * P],
                                rhs=qt[doff:doff + DH, :],
                                start=True, stop=True)
                        if PROBE == "smallexp":
                            nc.scalar.activation(
                                attn_flat[:, tau * 512:tau * 512 + 512],
                                st[:, 0:1, :], AF.Exp, scale=-SCALE)
                        else:
                            nc.scalar.activation(
                                attn_flat[:, tau * 512:(tau + sz) * 512],
                                st[:, 0:sz, :], AF.Exp, scale=-SCALE)
                        tau += sz
                        if g in hooks:
                            hooks[g]()
                        elif fillers:
                            fillers.pop(0)()
                    for f_ in fillers:   # leftovers run inline after the phase
                        f_()
                    del fillers[:]
                    return attn

                # ---------- per-rep schedule ----------
                pend_avs = []            # single-head AV closures, 1 phase back
                onorm_by_qc = {}
                xq_cur = load_xq(0)
                qt_next = None

                for qc in range(QC_RUN):
                    onorm = onorm_pool.tile([P, DT, 512], bf16, tag="onrm",
                                            name="onorm")
                    onorm_by_qc[qc] = onorm
                    for m in range(DT):
                        if m == 0 and qc == 0:
                            # serial head: kproj(m0), vproj(all), qproj(m0,0)
                            for t in range(TC):
                                for s_ in wproj_unit(
                                        wk_sb, bk_sb, 0, x1_sb, t * 512,
                                        k_sb[:, 0, t * 512:(t + 1) * 512]):
                                    s_()
                            for tt in range(KT):
                                for s_ in vproj_unit(tt):
                                    s_()
                            qt = q_pool.tile([P, 512], bf16, tag="qt",
                                             name="qt0")
                            for s_ in wproj_unit(wq_sb, bq_sb, 0, xq_cur, 0,
                                                 qt):
                                s_()
                        else:
                            qt = qt_next

                        # ---- fillers for this scores phase ----
                        fillers = []
                        # qproj of the NEXT (m, qc) first (its affine must
                        # land before the next phase's first score MM)
                        nm, nqc = (m + 1, qc) if m < DT - 1 else (0, qc + 1)
                        if nqc < QC_RUN:
                            if nm == 0:
                                xq_cur = load_xq(nqc)
                            qt_next = q_pool.tile([P, 512], bf16, tag="qt",
                                                  name="qtn")
                            fillers.extend(
                                wproj_unit(wq_sb, bq_sb, nm, xq_cur, 0,
                                           qt_next))
                        if qc == 0:
                            if m < DT - 1:
                                for t in range(TC):
                                    fillers.extend(wproj_unit(
                                        wk_sb, bk_sb, m + 1, x1_sb, t * 512,
                                        k_sb[:, m + 1, t * 512:(t + 1) * 512]))
                        else:
                            # outproj(qc-1) rides m1/m2 gaps: by then its
                            # onorm is fully written (m0's hooks still write
                            # onorm[., 3, :] of the previous qc)
                            if m == 1:
                                for jt in range(0, 4):
                                    fillers.append(
                                        outproj_unit(qc - 1, jt, onorm_by_qc[qc - 1]))
                            elif m == 2:
                                for jt in range(4, JT):
                                    fillers.append(
                                        outproj_unit(qc - 1, jt, onorm_by_qc[qc - 1]))

                        # ---- AV hooks: previous (m,qc)'s two heads ----
                        hooks = {}
                        if pend_avs:
                            hooks[1] = pend_avs.pop(0)
                            hooks[3] = pend_avs.pop(0)

                        attn = scores_exp(m, qt, fillers, hooks)
                        pend_avs.append(av_head(m, 0, attn, onorm))
                        pend_avs.append(av_head(m, 1, attn, onorm))

                # tail: last AV pair + out projection of qc3
                for run in pend_avs:
                    run()
                pend_avs = []
                for jt in range(JT):
                    outproj_unit(QC_RUN - 1, jt, onorm_by_qc[QC_RUN - 1])()

    nc.compile()
    return nc


def make_in_maps(x0, x1, x2, Wq, bq, Wk, bk, Wv, bv, Wp, bp):
    """Host-side shard prep: per-core transposed bf16 views."""
    bf = ml_dtypes.bfloat16
    zeros_bp = np.zeros_like(bp)
    xts = []
    for b in range(B):
        xts.append(
            (
                np.ascontiguousarray(x0[b].T).astype(bf),
                np.ascontiguousarray(x1[b].T).astype(bf),
                np.ascontiguousarray(x2[b].T).astype(bf),
            )
        )
    gparts = []
    for g in range(2):
        sl = slice(g * DG, (g + 1) * DG)
        bp_g = bp if g == 0 else zeros_bp
        gparts.append(
            {
                "wqt": np.ascontiguousarray(Wq[sl, :].T).astype(bf),
                "wkt": np.ascontiguousarray(Wk[sl, :].T).astype(bf),
                "wvt": np.ascontiguousarray(Wv[sl, :].T).astype(bf),
                "wpt": np.ascontiguousarray(Wp[:, sl].T).astype(bf),
                "bqr": np.ascontiguousarray(bq[sl].reshape(4, P).T).astype(np.float32),
                "bkr": np.ascontiguousarray(bk[sl].reshape(4, P).T).astype(np.float32),
                "bvr": bv[sl].reshape(1, DG).astype(bf),
                "bpr": np.ascontiguousarray(bp_g.reshape(8, P).T).astype(np.float32),
            }
        )
    in_maps = []
    for c in range(NCORES):
        b, g = c // 2, c % 2
        x0t_b, x1t_b, x2t_b = xts[b]
        m = {"x0t": x0t_b, "x1t": x1t_b, "x2t": x2t_b}
        m.update(gparts[g])
        in_maps.append(m)
    return in_maps


def assemble(results):
    out = np.empty((B, S, DIM), np.float32)
    for b in range(B):
        yp = results[2 * b]["y"] + results[2 * b + 1]["y"]
        out[b] = yp.T
    return out


def kernel(**inputs):
    from concourse.bass_utils import run_bass_kernel_spmd

    if "nc" not in _CACHE:
        _CACHE["nc"] = build_nc()
    nc = _CACHE["nc"]
    in_maps = make_in_maps(**inputs)
    res = run_bass_kernel_spmd(nc, in_maps, list(range(NCORES)))
    return assemble([r for r in res.results])
